# revision 3
# baseline (speedup 1.0000x reference)
"""Trainium2 Bass kernel for nn_BasicS2ConvV2.

out[b,d,p,r] = sum_{c,k,a} Wfull[d,c,r,k,a] * x[b,c,k,p,a]
with Wfull gathered on host from the 36 free params (tiny), and the
per-core contraction o[dr=192, p=4096] = WT[cka,192]^T @ x[cka,4096]
run on 8 NeuronCores, data-parallel over b.

Flipped (p-stationary) schedule: the stationary operand is an
x-tile [k=128, p=128] (always the full 128x128 PE array) and the
moving operand is WT[k=128, dr=192].  Unlike the dr-stationary
schedule (dr=192 forces a full-width pass plus a half-utilized
64-wide pass = 2x the moving cycles), every pass here runs at full
array utilization: 640 matmuls x 192 moving cols ~= 123k PE cycles.

x is shipped as float8 e3m4 (4 mantissa bits): halves HBM traffic
(10.5MB/core, ~29us) so DMA stays well under the PE time, and makes
the stationary FWL weight-load 4x (fully hidden).  W stays fp16 as
the moving operand; measured end-to-end rel err ~1.2e-2 (gate 2e-2).
"""

import numpy as np
import ml_dtypes

B, C, KS, P, A = 8, 16, 13, 4096, 12
D, R = 16, 12
CKA = C * KS * A          # 2496
KT = 20                   # contraction tiles of 128 (2560 padded)
CKA_PAD = KT * 128
DR = D * R                # 192
UW = 256                  # p-width per DMA unit (2 stationary blocks)
NU = P // UW              # 16 units

MMDT = "f8e3"             # x dtype: "f8e3" | "bf16"

_cache = {}


def _emit_body(nc, xs, wtile, o, x_dt, pspool, xpool, opool, reps):
    import concourse.mybir as mybir

    for u in [u for _ in range(reps) for u in range(NU)]:
        xt = xpool.tile([128, KT, UW], x_dt, tag="xt")
        xt_f = xt[:].rearrange("q t p -> q (t p)")
        xs_f = xs[u].rearrange("q t p -> q (t p)")
        # two half-unit DMAs on separate queues
        W_ = KT * UW // 2
        nc.scalar.dma_start(xt_f[:, 0:W_], xs_f[:, 0:W_])
        nc.sync.dma_start(xt_f[:, W_:2 * W_], xs_f[:, W_:2 * W_])
        for m in range(2):
            ps = pspool.tile([128, DR], mybir.dt.float32, tag=f"ps{m}")
            for t in range(KT):
                nc.tensor.matmul(
                    ps[:], xt[:, t, m * 128:(m + 1) * 128],
                    wtile[:, t, :],
                    start=(t == 0), stop=(t == KT - 1),
                )
            ot = opool.tile([128, DR], mybir.dt.float16, tag=f"ot{m}")
            nc.vector.tensor_copy(ot[:], ps[:])
            nc.sync.dma_start(o[u, m], ot[:])


def _build_program(mmdt, reps=1, loop_n=0, internal_io=False):
    import concourse.bacc as bacc
    import concourse.mybir as mybir
    from concourse.tile import TileContext
    from contextlib import nullcontext

    x_dt = {"f8e3": mybir.dt.float8e3, "bf16": mybir.dt.bfloat16}[mmdt]
    w_dt = mybir.dt.float16

    nc = bacc.Bacc("TRN2", target_bir_lowering=False, debug=False)
    if internal_io:
        # Timing-probe build: no host I/O traffic; data is device garbage.
        xs = nc.dram_tensor("xs", [NU, 128, KT, UW], x_dt).ap()
        wt = nc.dram_tensor("wt", [128, KT, DR], w_dt).ap()
        o = nc.dram_tensor("o", [NU, 2, 128, DR], mybir.dt.float16).ap()
        dume = nc.declare_dram_parameter(
            "dume", [1, 8], mybir.dt.float32, isOutput=True)
    else:
        xs = nc.declare_dram_parameter(
            "xs", [NU, 128, KT, UW], x_dt, isOutput=False)
        wt = nc.declare_dram_parameter(
            "wt", [128, KT, DR], w_dt, isOutput=False)
        o = nc.declare_dram_parameter(
            "o", [NU, 2, 128, DR], mybir.dt.float16, isOutput=True)

    with TileContext(nc) as tc:
        with (
            tc.tile_pool(name="wpool", bufs=1) as wpool,
            tc.tile_pool(name="xpool", bufs=6) as xpool,
            tc.tile_pool(name="opool", bufs=4) as opool,
            tc.tile_pool(name="pspool", bufs=4, space="PSUM") as pspool,
        ):
            # All weights resident in one tile, one DMA (2560 x 192 < 1MB)
            wtile = wpool.tile([128, KT, DR], w_dt)
            nc.sync.dma_start(wtile[:], wt[:])

            loop_cm = tc.For_i(0, loop_n, 1) if loop_n else nullcontext()
            with loop_cm:
                _emit_body(nc, xs, wtile, o, x_dt, pspool, xpool, opool, reps)

            if internal_io:
                dtile = opool.tile([1, 8], mybir.dt.float32, tag="dume")
                nc.any.memset(dtile[:], 1.0)
                nc.sync.dma_start(dume[:], dtile[:])

    nc.compile()
    return nc


def _get_program(mmdt):
    if mmdt not in _cache:
        _cache[mmdt] = _build_program(mmdt)
    return _cache[mmdt]


def _x_np_dtype(mmdt):
    return ml_dtypes.float8_e3m4 if mmdt == "f8e3" else ml_dtypes.bfloat16


def _prep_inputs(x, W, idx_map, tivr, tir, mmdt):
    """Host prep: weight gather + relayout to sequential-DMA order."""
    x_dt = _x_np_dtype(mmdt)

    Wm = W[:, :, idx_map].reshape(D, C, KS, A)
    Wfull = Wm[:, :, tivr[:, :, None], tir[:, None, :]]       # [d,c,r,k,a]
    WT = Wfull.transpose(1, 3, 4, 0, 2).reshape(CKA, DR)
    WT_pad = np.zeros((CKA_PAD, DR), dtype=np.float16)
    WT_pad[:CKA] = WT.astype(np.float16)
    # [2560, DR] -> [128(q), KT(t), DR]
    wt_q = np.ascontiguousarray(
        WT_pad.reshape(KT, 128, DR).transpose(1, 0, 2))

    # x[b,c,k,p,a] -> [b, (c,k,a), p] -> [b, NU(u), 128(q), KT(t), UW]
    xt = np.ascontiguousarray(x.transpose(0, 1, 2, 4, 3)).reshape(B, CKA, P)
    xs_pad = np.zeros((B, CKA_PAD, P), dtype=x_dt)
    xs_pad[:, :CKA] = xt.astype(x_dt)
    xs_q = np.ascontiguousarray(
        xs_pad.reshape(B, KT, 128, NU, UW).transpose(0, 3, 2, 1, 4))
    return xs_q, wt_q


def kernel(x, W, idx_map, trace_idxv_rot, trace_idx_rot):
    from concourse.bass_utils import run_bass_kernel_spmd

    x = np.asarray(x)
    W = np.asarray(W, dtype=np.float32)
    idx_map = np.asarray(idx_map)
    tivr = np.asarray(trace_idxv_rot)
    tir = np.asarray(trace_idx_rot)

    xs_q, wt_q = _prep_inputs(x, W, idx_map, tivr, tir, MMDT)

    nc = _get_program(MMDT)
    in_maps = [{"xs": xs_q[b], "wt": wt_q} for b in range(B)]
    res = run_bass_kernel_spmd(nc, in_maps, list(range(B)))

    out = np.empty((B, D, P, R), dtype=np.float32)
    for b in range(B):
        # o rows are p-major: [u, m, 128] = p, cols are dr = d*R + r
        oraw = np.asarray(res.results[b]["o"]).astype(np.float32)
        out[b] = oraw.reshape(P, D, R).transpose(1, 0, 2)
    return out


# revision 8
# speedup vs baseline: 1.0420x; 1.0420x over previous
"""Trainium2 Bass kernel for nn_BasicS2ConvV2.

out[b,d,p,r] = sum_{c,k,a} Wfull[d,c,r,k,a] * x[b,c,k,p,a]
with Wfull gathered on host from the 36 free params (tiny), and the
per-core contraction o[dr=192, p=4096] = WT[cka,192]^T @ x[cka,4096]
run on 8 NeuronCores, data-parallel over b.

Flipped (p-stationary) schedule: the stationary operand is an
x-tile [k=128, p=128] (always the full 128x128 PE array) and the
moving operand is WT[k=128, dr=192].  Unlike the dr-stationary
schedule (dr=192 forces a full-width pass plus a half-utilized
64-wide pass = 2x the moving cycles), every pass here runs at full
array utilization: ~123k PE stream-cycles, the floor for this GEMM
at bf16 stream rate.

x is shipped as float8 e3m4 (4 mantissa bits): halves HBM traffic
(10.5MB/core, ~29us) so DMA stays well under the PE time, and makes
the stationary FWL weight-load 4x (fully hidden).  W stays fp16 as
the moving operand; measured end-to-end rel err ~1.3e-2 (gate 2e-2).

The contraction is 2496 = 19.5 k-tiles; the two 64-row halves of the
padded 20th tile (p-block m=0 / m=1 of each unit) are issued as
row-tiled concurrent matmuls (tile_position rows 0-63 / 64-127), so
the pad tile costs one 192-col pass per unit instead of two.  Host
prep duplicates the last 64 k-rows of x and W into the pad rows to
feed the second row-tile.
"""

import numpy as np
import ml_dtypes

B, C, KS, P, A = 8, 16, 13, 4096, 12
D, R = 16, 12
CKA = C * KS * A          # 2496
KT = 20                   # contraction tiles of 128 (2560 padded)
KF = KT - 1               # full k-tiles; tile 19 is the split half-tile
CKA_PAD = KT * 128
DR = D * R                # 192
UW = 256                  # p-width per DMA unit (2 stationary blocks)
NU = P // UW              # 16 units
WQ = 4                    # weight DMA quarter-tiles
NWARM = 16                # PE warmup matmuls issued under the DMA head

MMDT = "f8e3"             # x dtype: "f8e3" | "bf16"

_cache = {}


def _emit_body(nc, xs, wq, o, x_dt, pspool, xpool, opool, reps):
    import concourse.mybir as mybir

    def wt(t):
        return wq[t // (KT // WQ)][:, t % (KT // WQ), :]

    for u in [u for _ in range(reps) for u in range(NU)]:
        xh = []
        for m in range(2):
            xt = xpool.tile([128, KT, 128], x_dt, tag=f"xt{m}")
            eng = nc.scalar if m == 0 else nc.sync
            eng.dma_start(xt[:].rearrange("q t p -> q (t p)"),
                          xs[u, m].rearrange("q t p -> q (t p)"))
            xh.append(xt)
        ps = [pspool.tile([128, DR], mybir.dt.float32, tag=f"ps{m}",
                          name=f"ps{m}")
              for m in range(2)]
        # m=0: full tiles then the low half of the pad tile ...
        for t in range(KF):
            nc.tensor.matmul(ps[0][:], xh[0][:, t, :], wt(t),
                             start=(t == 0), stop=False)
        nc.tensor.matmul(ps[0][:], xh[0][0:64, KF, :], wt(KF)[0:64],
                         start=False, stop=True, tile_position=(0, 0))
        # ... m=1: high half of the pad tile first (concurrent with the
        # low half above -- different PE row groups), then full tiles.
        nc.tensor.matmul(ps[1][:], xh[1][64:128, KF, :], wt(KF)[64:128],
                         start=True, stop=False, tile_position=(64, 0))
        for t in range(KF):
            nc.tensor.matmul(ps[1][:], xh[1][:, t, :], wt(t),
                             start=False, stop=(t == KF - 1))
        for m in range(2):
            ot = opool.tile([128, DR], mybir.dt.float16, tag=f"ot{m}")
            nc.vector.tensor_copy(ot[:], ps[m][:])
            eng = nc.scalar if (u + m) % 2 else nc.sync
            eng.dma_start(o[u, m], ot[:])


def _build_program(mmdt, reps=1, loop_n=0, internal_io=False):
    import concourse.bacc as bacc
    import concourse.mybir as mybir
    from concourse.tile import TileContext
    from contextlib import nullcontext

    x_dt = {"f8e3": mybir.dt.float8e3, "bf16": mybir.dt.bfloat16}[mmdt]
    w_dt = mybir.dt.float16

    nc = bacc.Bacc("TRN2", target_bir_lowering=False, debug=False)
    if internal_io:
        # Timing-probe build: no host I/O traffic; data is device garbage.
        xs = nc.dram_tensor("xs", [NU, 2, 128, KT, 128], x_dt).ap()
        wtd = nc.dram_tensor("wt", [WQ, 128, KT // WQ, DR], w_dt).ap()
        o = nc.dram_tensor("o", [NU, 2, 128, DR], mybir.dt.float16).ap()
        dume = nc.declare_dram_parameter(
            "dume", [1, 8], mybir.dt.float32, isOutput=True)
    else:
        xs = nc.declare_dram_parameter(
            "xs", [NU, 2, 128, KT, 128], x_dt, isOutput=False)
        wtd = nc.declare_dram_parameter(
            "wt", [WQ, 128, KT // WQ, DR], w_dt, isOutput=False)
        o = nc.declare_dram_parameter(
            "o", [NU, 2, 128, DR], mybir.dt.float16, isOutput=True)

    with TileContext(nc) as tc:
        with (
            tc.tile_pool(name="wpool", bufs=1) as wpool,
            tc.tile_pool(name="xpool", bufs=6) as xpool,
            tc.tile_pool(name="opool", bufs=4) as opool,
            tc.tile_pool(name="pspool", bufs=3, space="PSUM") as pspool,
            tc.tile_pool(name="wpspool", bufs=1, space="PSUM") as wpspool,
        ):
            # Weights resident in 4 quarter-tiles on their own queue so
            # the first matmuls only wait for quarter 0 (~0.7us).
            wq = []
            for q in range(WQ):
                w_t = wpool.tile([128, KT // WQ, DR], w_dt, tag=f"w{q}")
                nc.gpsimd.dma_start(w_t[:], wtd[q])
                wq.append(w_t)

            # PE warmup under the DMA head: keeps the PE-busy HAM window
            # filling while the first x/W tiles land.
            wrm = wpool.tile([128, 128], x_dt, tag="wrm")
            nc.any.memset(wrm[:], 0.25)
            wps = wpspool.tile([128, 64], mybir.dt.float32, tag="wps")
            for _ in range(NWARM):
                nc.tensor.matmul(wps[:], wrm[:], wrm[:, 0:64], start=True,
                                 stop=True)

            loop_cm = tc.For_i(0, loop_n, 1) if loop_n else nullcontext()
            with loop_cm:
                _emit_body(nc, xs, wq, o, x_dt, pspool, xpool, opool, reps)

            if internal_io:
                dtile = opool.tile([1, 8], mybir.dt.float32, tag="dume")
                nc.any.memset(dtile[:], 1.0)
                nc.sync.dma_start(dume[:], dtile[:])

    nc.compile()
    return nc


def _get_program(mmdt):
    if mmdt not in _cache:
        _cache[mmdt] = _build_program(mmdt)
    return _cache[mmdt]


def _x_np_dtype(mmdt):
    return ml_dtypes.float8_e3m4 if mmdt == "f8e3" else ml_dtypes.bfloat16


def _prep_inputs(x, W, idx_map, tivr, tir, mmdt):
    """Host prep: weight gather + relayout to sequential-DMA order."""
    x_dt = _x_np_dtype(mmdt)

    Wm = W[:, :, idx_map].reshape(D, C, KS, A)
    Wfull = Wm[:, :, tivr[:, :, None], tir[:, None, :]]       # [d,c,r,k,a]
    WT = Wfull.transpose(1, 3, 4, 0, 2).reshape(CKA, DR)
    WT_pad = np.zeros((CKA_PAD, DR), dtype=np.float16)
    WT_pad[:CKA] = WT.astype(np.float16)
    WT_pad[CKA:] = WT_pad[CKA - 64:CKA]   # pad rows = dup of last half-tile
    # [2560, DR] -> [128(q), KT(t), DR] -> [WQ, 128, KT/WQ, DR]
    wt_q = np.ascontiguousarray(
        WT_pad.reshape(KT, 128, DR).transpose(1, 0, 2)
        .reshape(128, WQ, KT // WQ, DR).transpose(1, 0, 2, 3))

    # x[b,c,k,p,a] -> [b, (c,k,a), p] -> [b, NU(u), 2(m), 128(q), KT(t), 128]
    xt = np.ascontiguousarray(x.transpose(0, 1, 2, 4, 3)).reshape(B, CKA, P)
    xs_pad = np.zeros((B, CKA_PAD, P), dtype=x_dt)
    xs_pad[:, :CKA] = xt.astype(x_dt)
    xs_pad[:, CKA:] = xs_pad[:, CKA - 64:CKA]
    xs_q = np.ascontiguousarray(
        xs_pad.reshape(B, KT, 128, NU, 2, 128).transpose(0, 3, 4, 2, 1, 5))
    return xs_q, wt_q


def kernel(x, W, idx_map, trace_idxv_rot, trace_idx_rot):
    from concourse.bass_utils import run_bass_kernel_spmd

    x = np.asarray(x)
    W = np.asarray(W, dtype=np.float32)
    idx_map = np.asarray(idx_map)
    tivr = np.asarray(trace_idxv_rot)
    tir = np.asarray(trace_idx_rot)

    xs_q, wt_q = _prep_inputs(x, W, idx_map, tivr, tir, MMDT)

    nc = _get_program(MMDT)
    in_maps = [{"xs": xs_q[b], "wt": wt_q} for b in range(B)]
    res = run_bass_kernel_spmd(nc, in_maps, list(range(B)))

    out = np.empty((B, D, P, R), dtype=np.float32)
    for b in range(B):
        # o rows are p-major: [u, m, 128] = p, cols are dr = d*R + r
        oraw = np.asarray(res.results[b]["o"]).astype(np.float32)
        out[b] = oraw.reshape(P, D, R).transpose(1, 0, 2)
    return out
